# revision 1
# baseline (speedup 1.0000x reference)
"""Trainium2 Bass kernel for CusMultiHeadAttention.

Shapes (hardcoded): x (4,1024,1024) f32, bias (4,16,1024,1024) f32,
attention_mask (4,1024) i32, Wq/Wk/Wv (1024,1024), Wo (1024,1024), bo (1024,).

Sharding: 8 cores = 4 batches x 2 head-groups (8 heads each).
Wq/Wk/Wv column-parallel, Wo row-parallel (host sums the pair partials + bo).

Per-core pipeline (all "transposed" orientation, no on-device transposes):
  xT = x[b].T (host)                     -> SBUF (c_in on partitions)
  qT = (Wq'/8)^T @ xT, kT = Wk'^T @ xT   (feature on partitions, seq free)
  v  = x[b] @ Wv'                        (seq on partitions, feature free)
  v_aug[h] = [v[h] * mask | mask]        (mask folded into V + ones-column)
  sT[h,kt] = kT[h,kt].T @ qT[h] + biasT  (k on partitions, q free; biasT from host)
  pT = exp(sT)                           (no max subtraction; scores are O(5))
  o_aug[h] = sum_kt v_aug[h,kt].T @ pT[h,kt]   (rows 0..63 = o.T, row 64 = denom)
  oT[h] = o_aug[0:64] * bcast(1/denom)   (denom recip at p64 hops to p0 via a
                                          tiny SBUF DMA, then gpsimd broadcast)
  outp = sum_h oT[h].T @ Wo'[h]          (q on partitions) -> DRAM partial
"""

import sys

if "/opt/trn_rl_repo" not in sys.path:
    sys.path.insert(0, "/opt/trn_rl_repo")

import math
from contextlib import ExitStack

import numpy as np

import concourse.mybir as mybir
import concourse.tile as tile
from concourse import bacc
from concourse.alu_op_type import AluOpType
from concourse.bass_utils import run_bass_kernel_spmd

B, S, C_IN = 4, 1024, 1024
N_HEAD, C = 16, 64
N_CORES = 8
HG = 8  # heads per core
F = HG * C  # 512 local features
P = 128
KT = C_IN // P  # 8 contraction tiles for projections
ST = S // P  # 8 seq tiles
VW = C + 1  # 65: v columns + ones-column

f32 = mybir.dt.float32
bf16 = mybir.dt.bfloat16


def build_program(taps=False):
    nc = bacc.Bacc("TRN2", target_bir_lowering=False, debug=False,
                   num_devices=N_CORES)

    xT = nc.dram_tensor("xT", (C_IN, S), bf16, kind="ExternalInput").ap()
    wq = nc.dram_tensor("wq", (C_IN, F), bf16, kind="ExternalInput").ap()
    wk = nc.dram_tensor("wk", (C_IN, F), bf16, kind="ExternalInput").ap()
    wv = nc.dram_tensor("wv", (C_IN, F), bf16, kind="ExternalInput").ap()
    wo = nc.dram_tensor("wo", (F, C_IN), bf16, kind="ExternalInput").ap()
    biasT = nc.dram_tensor("biasT", (HG, S, S), bf16, kind="ExternalInput").ap()
    maskf = nc.dram_tensor("maskf", (S,), f32, kind="ExternalInput").ap()
    outp = nc.dram_tensor("outp", (S, C_IN), f32, kind="ExternalOutput").ap()
    if taps:
        dbg_qT = nc.dram_tensor("dbg_qT", (P, F // P, S), bf16,
                                kind="ExternalOutput").ap()
        dbg_kT = nc.dram_tensor("dbg_kT", (P, F // P, S), bf16,
                                kind="ExternalOutput").ap()
        dbg_v = nc.dram_tensor("dbg_v", (P, ST, HG * VW), bf16,
                               kind="ExternalOutput").ap()
        dbg_oT = nc.dram_tensor("dbg_oT", (C, HG, S), bf16,
                                kind="ExternalOutput").ap()
        dbg_pt = nc.dram_tensor("dbg_pt", (P, S), bf16,
                                kind="ExternalOutput").ap()
        dbg_rc = nc.dram_tensor("dbg_rc", (1, S), f32,
                                kind="ExternalOutput").ap()
        dbg_rcb = nc.dram_tensor("dbg_rcb", (C, S), f32,
                                 kind="ExternalOutput").ap()

    with tile.TileContext(nc) as tc:
        with ExitStack() as ctx:
            persist = ctx.enter_context(tc.tile_pool(name="persist", bufs=1))
            mask_sb = persist.tile([P, ST], f32)
            nc.sync.dma_start(mask_sb[:], maskf.rearrange("(t p) -> p t", p=P))
            ones_sb = persist.tile([P, HG, 1], f32)
            nc.vector.memset(ones_sb[:], 1.0)
            v_sb = persist.tile([P, ST, HG * VW], bf16)
            qT_sb = persist.tile([P, F // P, S], bf16)
            kT_sb = persist.tile([P, F // P, S], bf16)
            # per-head rows at partitions 0..63 (matmul needs equal base
            # partition for lhsT and rhs; oT lives at partitions 0..63)
            wo_sb = persist.tile([C, HG, C_IN], bf16)
            nc.sync.dma_start(
                wo_sb[:], wo.rearrange("(h j) n -> j h n", j=C))

            # ---- phase A: projections (xT/wq/wk/wv live only here) ----
            with tc.tile_pool(name="phaseA", bufs=1) as pa, \
                 tc.tile_pool(name="psProj", bufs=2, space="PSUM") as psProj, \
                 tc.tile_pool(name="psV", bufs=2, space="PSUM") as psV:
                xT_sb = pa.tile([P, KT, S], bf16)
                wq_sb = pa.tile([P, KT, F], bf16)
                wk_sb = pa.tile([P, KT, F], bf16)
                wv_sb = pa.tile([P, KT, F], bf16)
                for kt in range(KT):
                    nc.sync.dma_start(
                        xT_sb[:, kt, :],
                        xT[kt * P:(kt + 1) * P, :])
                    nc.sync.dma_start(
                        wq_sb[:, kt, :],
                        wq[kt * P:(kt + 1) * P, :])
                    nc.sync.dma_start(
                        wk_sb[:, kt, :],
                        wk[kt * P:(kt + 1) * P, :])
                    nc.sync.dma_start(
                        wv_sb[:, kt, :],
                        wv[kt * P:(kt + 1) * P, :])

                # qT, kT: (feature on partitions, seq free)
                for mt in range(F // P):
                    for w_sb, dst in ((wq_sb, qT_sb), (wk_sb, kT_sb)):
                        ps = psProj.tile([P, S], f32, name="ps_proj")
                        for nh in range(2):
                            for kt in range(KT):
                                nc.tensor.matmul(
                                    ps[:, nh * 512:(nh + 1) * 512],
                                    w_sb[:, kt, mt * P:(mt + 1) * P],
                                    xT_sb[:, kt, nh * 512:(nh + 1) * 512],
                                    start=(kt == 0), stop=(kt == KT - 1))
                        nc.scalar.copy(dst[:, mt, :], ps[:])

                # v natural (seq on partitions), mask+ones folded
                for mt in range(ST):
                    psv = psV.tile([P, F], f32, name="psv")
                    for kt in range(KT):
                        nc.tensor.matmul(
                            psv[:],
                            xT_sb[:, kt, mt * P:(mt + 1) * P],
                            wv_sb[:, kt, :],
                            start=(kt == 0), stop=(kt == KT - 1))
                    m_col = mask_sb[:, mt:mt + 1]
                    v_view = v_sb[:, mt, :].rearrange("p (h c) -> p h c", c=VW)
                    nc.vector.tensor_scalar_mul(
                        v_view[:, :, 0:C],
                        psv.rearrange("p (h c) -> p h c", c=C), m_col)
                    nc.vector.tensor_scalar_mul(
                        v_view[:, :, C:C + 1], ones_sb[:], m_col)

            # ---- phase B: attention ----
            oT_pool = ctx.enter_context(tc.tile_pool(name="oTp", bufs=1))
            oT_sb = oT_pool.tile([C, HG, S], bf16)
            with tc.tile_pool(name="bias", bufs=8) as bias_pool, \
                 tc.tile_pool(name="pT", bufs=4) as pT_pool, \
                 tc.tile_pool(name="rc", bufs=2) as rc_pool, \
                 tc.tile_pool(name="rc0", bufs=2) as rc0_pool, \
                 tc.tile_pool(name="rcb", bufs=2) as rcb_pool, \
                 tc.tile_pool(name="psS", bufs=2, space="PSUM") as psS, \
                 tc.tile_pool(name="psO", bufs=2, space="PSUM") as psO:

                for h in range(HG):
                    po = (h % 2) * C  # partition offset of head in qT/kT
                    mt_h = h // 2
                    kT_h = kT_sb[po:po + C, mt_h, :]
                    qT_h = qT_sb[po:po + C, mt_h, :]
                    oaps = psO.tile([VW, S], f32, name="oaug")
                    for kt in range(ST):
                        ps_s = psS.tile([P, S], f32, name="ps_s")
                        for nh in range(2):
                            nc.tensor.matmul(
                                ps_s[:, nh * 512:(nh + 1) * 512],
                                kT_h[:, kt * P:(kt + 1) * P],
                                qT_h[:, nh * 512:(nh + 1) * 512],
                                start=True, stop=True)
                        bt = bias_pool.tile([P, S], bf16, name="bt")
                        nc.sync.dma_start(bt[:],
                                          biasT[h, kt * P:(kt + 1) * P, :])
                        nc.vector.tensor_tensor(ps_s[:], ps_s[:], bt[:],
                                                AluOpType.add)
                        pt = pT_pool.tile([P, S], bf16, name="pt")
                        nc.scalar.activation(pt[:], ps_s[:],
                                             mybir.ActivationFunctionType.Exp)
                        if taps and h == 0 and kt == 0:
                            nc.sync.dma_start(dbg_pt, pt[:])
                        for nh in range(2):
                            nc.tensor.matmul(
                                oaps[:, nh * 512:(nh + 1) * 512],
                                v_sb[:, kt, h * VW:(h + 1) * VW],
                                pt[:, nh * 512:(nh + 1) * 512],
                                start=(kt == 0), stop=(kt == ST - 1))
                    # denom row sits at psum partition 64: copy to SBUF,
                    # hop to p0 via tiny SBUF DMA, recip, broadcast.
                    rc = rc_pool.tile([P, S], f32, name="rc")
                    nc.scalar.copy(rc[C:C + 1, :], oaps[C:C + 1, :])
                    rc0 = rc0_pool.tile([1, S], f32, name="rc0")
                    nc.sync.dma_start(rc0[:], rc[C:C + 1, :])
                    rcv = rc0_pool.tile([1, S], f32, name="rcv", tag="rcv")
                    nc.vector.reciprocal_approx_fast(rcv[:], rc0[:])
                    rcb = rcb_pool.tile([C, S], f32, name="rcb")
                    nc.gpsimd.partition_broadcast(rcb[:], rcv[:])
                    if taps and h == 0:
                        nc.sync.dma_start(dbg_rc[:], rcv[:])
                        nc.sync.dma_start(dbg_rcb[:], rcb[:])
                    nc.vector.tensor_mul(oT_sb[:, h, :], oaps[0:C, :], rcb[:])

                if taps:
                    nc.sync.dma_start(dbg_qT, qT_sb[:])
                    nc.sync.dma_start(dbg_kT, kT_sb[:])
                    nc.sync.dma_start(dbg_v, v_sb[:])
                    nc.sync.dma_start(dbg_oT, oT_sb[:])

            # ---- output projection (row-parallel partial) ----
            with tc.tile_pool(name="outsb", bufs=3) as out_pool, \
                 tc.tile_pool(name="psOut", bufs=2, space="PSUM") as psOut:
                for qt in range(ST):
                    for nh in range(2):
                        pso = psOut.tile([P, 512], f32, name="pso")
                        for h in range(HG):
                            nc.tensor.matmul(
                                pso[:],
                                oT_sb[:, h, qt * P:(qt + 1) * P],
                                wo_sb[:, h, nh * 512:(nh + 1) * 512],
                                start=(h == 0), stop=(h == HG - 1))
                        osb = out_pool.tile([P, 512], f32, name="osb")
                        nc.scalar.copy(osb[:], pso[:])
                        nc.sync.dma_start(
                            outp[qt * P:(qt + 1) * P,
                                 nh * 512:(nh + 1) * 512],
                            osb[:])

    nc.compile()
    return nc


def make_in_maps(x, bias, attention_mask, Wq, Wk, Wv, Wo):
    import ml_dtypes
    bf = ml_dtypes.bfloat16
    scale = 1.0 / math.sqrt(C)
    wq_scaled = (np.asarray(Wq) * scale).astype(bf)
    x = np.asarray(x)
    bias = np.asarray(bias)
    wk16 = np.asarray(Wk).astype(bf)
    wv16 = np.asarray(Wv).astype(bf)
    wo16 = np.asarray(Wo).astype(bf)
    in_maps = []
    for c in range(N_CORES):
        b, hg = c // 2, c % 2
        fs = slice(hg * F, (hg + 1) * F)
        in_maps.append({
            "xT": np.ascontiguousarray(x[b].T.astype(bf)),
            "wq": np.ascontiguousarray(wq_scaled[:, fs]),
            "wk": np.ascontiguousarray(wk16[:, fs]),
            "wv": np.ascontiguousarray(wv16[:, fs]),
            "wo": np.ascontiguousarray(wo16[fs, :]),
            "biasT": np.ascontiguousarray(
                bias[b, hg * HG:(hg + 1) * HG].transpose(0, 2, 1).astype(bf)),
            "maskf": np.asarray(attention_mask)[b].astype(np.float32),
        })
    return in_maps


_NC_CACHE = []


def get_program():
    if not _NC_CACHE:
        _NC_CACHE.append(build_program())
    return _NC_CACHE[0]


def run(in_maps, trace=False, **kw):
    nc = get_program()
    return run_bass_kernel_spmd(nc, in_maps, core_ids=list(range(N_CORES)),
                                trace=trace, **kw)


def kernel(x, bias, attention_mask, Wq, Wk, Wv, Wo, bo):
    in_maps = make_in_maps(x, bias, attention_mask, Wq, Wk, Wv, Wo)
    res = run(in_maps)
    out = np.empty((B, S, C_IN), dtype=np.float32)
    for b in range(B):
        out[b] = (res.results[2 * b]["outp"] + res.results[2 * b + 1]["outp"]
                  + np.asarray(bo).astype(np.float32))
    return out



# revision 6
# speedup vs baseline: 1.0192x; 1.0192x over previous
"""Trainium2 Bass kernel for CusMultiHeadAttention.

Shapes (hardcoded): x (4,1024,1024) f32, bias (4,16,1024,1024) f32,
attention_mask (4,1024) i32, Wq/Wk/Wv (1024,1024), Wo (1024,1024), bo (1024,).

Sharding: 8 cores = 4 batches x 2 head-groups (8 heads each).
Wq/Wk/Wv column-parallel, Wo row-parallel (host sums the pair partials + bo).

Per-core pipeline (all "transposed" orientation, no on-device transposes):
  xT = x[b].T (host)                     -> SBUF (c_in on partitions)
  qT = (Wq'/8)^T @ xT, kT = Wk'^T @ xT   (feature on partitions, seq free)
  v  = x[b] @ Wv'                        (seq on partitions, feature free)
  v_aug[h] = [v[h] * mask | mask]        (mask folded into V + ones-column)
  sT[h,t]  = kT[h,kt].T @ qT[h,nh]       ([128,512] PSUM half-tiles)
  pT = exp(sT) * expbT                   (exp on Act; host-precomputed
                                          exp(bias) folded in via bf16 DVE mul)
  o_aug[h] = sum_t v_aug[h,kt].T @ pT[h,t]   (rows 0..63 = o.T, row 64 = denom)
  oT[h] = o_aug[0:64] * bcast(1/denom)   (denom recip at p64 hops to p0 via a
                                          tiny SBUF DMA, then pool broadcast)
  head pairs (2j, 2j+1) stacked into oT_pack [128, 4, S] (odd head rows hop
  64.. via SBUF DMA) so the output projection contracts K=128:
  outp = sum_j oT_pack[:,j].T @ wo_pack[:,j]  -> DRAM partial (host adds + bo)
"""

import sys

if "/opt/trn_rl_repo" not in sys.path:
    sys.path.insert(0, "/opt/trn_rl_repo")

import math
from contextlib import ExitStack

import numpy as np

import concourse.mybir as mybir
import concourse.tile as tile
from concourse import bacc
from concourse.bass_utils import run_bass_kernel_spmd

B, S, C_IN = 4, 1024, 1024
N_HEAD, C = 16, 64
N_CORES = 8
HG = 8  # heads per core
F = HG * C  # 512 local features
P = 128
KT = C_IN // P  # 8 contraction tiles for projections
ST = S // P  # 8 seq tiles
VW = C + 1  # 65: v columns + ones-column
NT = 2 * ST  # 16 half-tiles (kt, nh) per head
LOOKAHEAD = 3  # S-tiles in flight ahead of the exp/mul/PV chain

f32 = mybir.dt.float32
bf16 = mybir.dt.bfloat16


def build_program():
    nc = bacc.Bacc("TRN2", target_bir_lowering=False, debug=False,
                   num_devices=N_CORES)

    xT = nc.dram_tensor("xT", (C_IN, S), bf16, kind="ExternalInput").ap()
    wq = nc.dram_tensor("wq", (C_IN, F), bf16, kind="ExternalInput").ap()
    wk = nc.dram_tensor("wk", (C_IN, F), bf16, kind="ExternalInput").ap()
    wv = nc.dram_tensor("wv", (C_IN, F), bf16, kind="ExternalInput").ap()
    # packed head pairs: rows 0:64 = head 2j, rows 64:128 = head 2j+1
    wo = nc.dram_tensor("wo", (P, HG // 2, C_IN), bf16,
                        kind="ExternalInput").ap()
    expbT = nc.dram_tensor("expbT", (HG, S, S), bf16,
                           kind="ExternalInput").ap()
    maskf = nc.dram_tensor("maskf", (S,), f32, kind="ExternalInput").ap()
    outp = nc.dram_tensor("outp", (S, C_IN), f32, kind="ExternalOutput").ap()

    with tile.TileContext(nc) as tc:
        with ExitStack() as ctx:
            persist = ctx.enter_context(tc.tile_pool(name="persist", bufs=1))
            mask_sb = persist.tile([P, ST], f32)
            ones_sb = persist.tile([P, HG, 1], f32)
            v_sb = persist.tile([P, ST, HG * VW], bf16)
            qT_sb = persist.tile([P, F // P, S], bf16)
            kT_sb = persist.tile([P, F // P, S], bf16)
            wo_sb = persist.tile([P, HG // 2, C_IN], bf16)
            oT_pack = persist.tile([P, HG // 2, S], bf16)
            oT_hi = persist.tile([C, HG // 2, S], bf16)

            # ---- input DMAs spread across the sync/vector/gpsimd queues ----
            xT_sb = persist.tile([P, KT, S], bf16)
            wq_sb = persist.tile([P, KT, F], bf16)
            wk_sb = persist.tile([P, KT, F], bf16)
            wv_sb = persist.tile([P, KT, F], bf16)
            nc.sync.dma_start(wq_sb[:], wq.rearrange("(kt p) f -> p kt f", p=P))
            nc.scalar.dma_start(wk_sb[:],
                                wk.rearrange("(kt p) f -> p kt f", p=P))
            for kt in range(KT):
                eng = nc.sync if kt % 2 == 0 else nc.scalar
                eng.dma_start(xT_sb[:, kt, :], xT[kt * P:(kt + 1) * P, :])
            nc.scalar.dma_start(wv_sb[:],
                                wv.rearrange("(kt p) f -> p kt f", p=P))
            nc.gpsimd.dma_start(wo_sb[:], wo)
            nc.gpsimd.dma_start(mask_sb[:],
                                maskf.rearrange("(t p) -> p t", p=P))
            nc.gpsimd.memset(ones_sb[:], 1.0)

            # exp(bias) head tiles, double-buffered, alternating queues
            expb_pool = ctx.enter_context(tc.tile_pool(name="expb", bufs=2))

            def expb_dma(h):
                t = expb_pool.tile([P, ST, S], bf16, name=f"expb{h}",
                                   tag="expb")
                eng = nc.sync if h % 2 == 0 else nc.scalar
                eng.dma_start(t[:],
                              expbT[h].rearrange("(kt p) s -> p kt s", p=P))
                return t

            expb_tiles = {h: expb_dma(h) for h in range(2)}

            # ---- phase P: projections ----
            with tc.tile_pool(name="psProj", bufs=2, space="PSUM") as psProj, \
                 tc.tile_pool(name="psV", bufs=2, space="PSUM") as psV:
                # qT, kT: (feature on partitions, seq free)
                for mt in range(F // P):
                    for w_sb, dst in ((wq_sb, qT_sb), (wk_sb, kT_sb)):
                        ps = psProj.tile([P, S], f32, name="ps_proj")
                        for nh in range(2):
                            for kt in range(KT):
                                nc.tensor.matmul(
                                    ps[:, nh * 512:(nh + 1) * 512],
                                    w_sb[:, kt, mt * P:(mt + 1) * P],
                                    xT_sb[:, kt, nh * 512:(nh + 1) * 512],
                                    start=(kt == 0), stop=(kt == KT - 1))
                        nc.vector.tensor_copy(dst[:, mt, :], ps[:])

                # v natural (seq on partitions), mask+ones folded
                for mt in range(ST):
                    psv = psV.tile([P, F], f32, name="psv")
                    for kt in range(KT):
                        nc.tensor.matmul(
                            psv[:],
                            xT_sb[:, kt, mt * P:(mt + 1) * P],
                            wv_sb[:, kt, :],
                            start=(kt == 0), stop=(kt == KT - 1))
                    m_col = mask_sb[:, mt:mt + 1]
                    v_view = v_sb[:, mt, :].rearrange("p (h c) -> p h c", c=VW)
                    nc.vector.tensor_scalar_mul(
                        v_view[:, :, 0:C],
                        psv.rearrange("p (h c) -> p h c", c=C), m_col)
                    nc.vector.tensor_scalar_mul(
                        v_view[:, :, C:C + 1], ones_sb[:], m_col)

            # ---- phase A: attention ----
            with tc.tile_pool(name="psS", bufs=4, space="PSUM") as psS, \
                 tc.tile_pool(name="psO", bufs=2, space="PSUM") as psO, \
                 tc.tile_pool(name="pe", bufs=4) as pe_pool, \
                 tc.tile_pool(name="pt", bufs=4) as pt_pool, \
                 tc.tile_pool(name="rcd", bufs=2) as rcd_pool, \
                 tc.tile_pool(name="rc0", bufs=2) as rc0_pool, \
                 tc.tile_pool(name="rcb", bufs=2) as rcb_pool:

                for h in range(HG):
                    po = (h % 2) * C  # partition offset of head in qT/kT
                    mt_h = h // 2
                    kT_h = kT_sb[po:po + C, mt_h, :]
                    qT_h = qT_sb[po:po + C, mt_h, :]
                    expb_sb = expb_tiles[h]
                    oaps = psO.tile([VW, S], f32, name="oaug")
                    s_tiles = [None] * NT

                    def chain(t):
                        kt, nh = t // 2, t % 2
                        pe_t = pe_pool.tile([P, 512], bf16, name="pe",
                                            tag="pe")
                        nc.scalar.activation(pe_t[:], s_tiles[t][:],
                                             mybir.ActivationFunctionType.Exp)
                        pt_t = pt_pool.tile([P, 512], bf16, name="pt",
                                            tag="pt")
                        nc.vector.tensor_mul(
                            pt_t[:], pe_t[:],
                            expb_sb[:, kt, nh * 512:(nh + 1) * 512])
                        nc.tensor.matmul(
                            oaps[:, nh * 512:(nh + 1) * 512],
                            v_sb[:, kt, h * VW:(h + 1) * VW],
                            pt_t[:],
                            start=(kt == 0), stop=(kt == ST - 1))

                    for t in range(NT + LOOKAHEAD):
                        if t < NT:
                            kt, nh = t // 2, t % 2
                            ps_t = psS.tile([P, 512], f32, name="ps_s",
                                            tag="ps_s")
                            s_tiles[t] = ps_t
                            nc.tensor.matmul(
                                ps_t[:],
                                kT_h[:, kt * P:(kt + 1) * P],
                                qT_h[:, nh * 512:(nh + 1) * 512],
                                start=True, stop=True)
                        if t >= LOOKAHEAD:
                            chain(t - LOOKAHEAD)

                    # prefetch exp(bias) for head h+2 into the freed slot
                    if h + 2 < HG:
                        expb_tiles[h + 2] = expb_dma(h + 2)

                    # denom row at psum partition 64: copy to SBUF, hop to
                    # p0 via tiny SBUF DMA, recip, broadcast to 64 rows.
                    rcd = rcd_pool.tile([P, S], f32, name="rcd")
                    nc.vector.tensor_copy(rcd[C:C + 1, :], oaps[C:C + 1, :])
                    rc0 = rc0_pool.tile([1, S], f32, name="rc0")
                    nc.sync.dma_start(rc0[:], rcd[C:C + 1, :])
                    rcv = rc0_pool.tile([1, S], f32, name="rcv", tag="rcv")
                    nc.vector.reciprocal_approx_fast(rcv[:], rc0[:])
                    rcb = rcb_pool.tile([C, S], f32, name="rcb")
                    nc.gpsimd.partition_broadcast(rcb[:], rcv[:])
                    j = h // 2
                    if h % 2 == 0:
                        nc.vector.tensor_mul(oT_pack[0:C, j, :],
                                             oaps[0:C, :], rcb[:])
                    else:
                        nc.vector.tensor_mul(oT_hi[:, j, :],
                                             oaps[0:C, :], rcb[:])
                        nc.gpsimd.dma_start(oT_pack[C:P, j, :],
                                            oT_hi[:, j, :])

            # ---- phase O: output projection (row-parallel partial) ----
            with tc.tile_pool(name="outsb", bufs=3) as out_pool, \
                 tc.tile_pool(name="psOut", bufs=2, space="PSUM") as psOut:
                for qt in range(ST):
                    for nh in range(2):
                        pso = psOut.tile([P, 512], f32, name="pso")
                        for j in range(HG // 2):
                            nc.tensor.matmul(
                                pso[:],
                                oT_pack[:, j, qt * P:(qt + 1) * P],
                                wo_sb[:, j, nh * 512:(nh + 1) * 512],
                                start=(j == 0), stop=(j == HG // 2 - 1))
                        osb = out_pool.tile([P, 512], f32, name="osb")
                        nc.scalar.copy(osb[:], pso[:])
                        nc.gpsimd.dma_start(
                            outp[qt * P:(qt + 1) * P,
                                 nh * 512:(nh + 1) * 512],
                            osb[:])

    nc.compile()
    return nc


def make_in_maps(x, bias, attention_mask, Wq, Wk, Wv, Wo):
    import ml_dtypes
    bf = ml_dtypes.bfloat16
    scale = 1.0 / math.sqrt(C)
    wq_scaled = (np.asarray(Wq) * scale).astype(bf)
    x = np.asarray(x)
    bias = np.asarray(bias)
    wk16 = np.asarray(Wk).astype(bf)
    wv16 = np.asarray(Wv).astype(bf)
    wo = np.asarray(Wo)
    in_maps = []
    for c in range(N_CORES):
        b, hg = c // 2, c % 2
        fs = slice(hg * F, (hg + 1) * F)
        # pack Wo rows as head pairs (2j, 2j+1) stacked along partitions
        wo_l = wo[fs, :].reshape(HG, C, C_IN)
        wo_pack = np.concatenate(
            [np.concatenate([wo_l[2 * j], wo_l[2 * j + 1]], axis=0)[None]
             for j in range(HG // 2)], axis=0)  # (4, 128, C_IN)
        wo_pack = np.ascontiguousarray(
            wo_pack.transpose(1, 0, 2)).astype(bf)  # (128, 4, C_IN)
        expb = np.exp(
            bias[b, hg * HG:(hg + 1) * HG].transpose(0, 2, 1)).astype(bf)
        in_maps.append({
            "xT": np.ascontiguousarray(x[b].T.astype(bf)),
            "wq": np.ascontiguousarray(wq_scaled[:, fs]),
            "wk": np.ascontiguousarray(wk16[:, fs]),
            "wv": np.ascontiguousarray(wv16[:, fs]),
            "wo": wo_pack,
            "expbT": np.ascontiguousarray(expb),
            "maskf": np.asarray(attention_mask)[b].astype(np.float32),
        })
    return in_maps


_NC_CACHE = []


def get_program():
    if not _NC_CACHE:
        _NC_CACHE.append(build_program())
    return _NC_CACHE[0]


def run(in_maps, trace=False, **kw):
    nc = get_program()
    return run_bass_kernel_spmd(nc, in_maps, core_ids=list(range(N_CORES)),
                                trace=trace, **kw)


def kernel(x, bias, attention_mask, Wq, Wk, Wv, Wo, bo):
    in_maps = make_in_maps(x, bias, attention_mask, Wq, Wk, Wv, Wo)
    res = run(in_maps)
    out = np.empty((B, S, C_IN), dtype=np.float32)
    for b in range(B):
        out[b] = (res.results[2 * b]["outp"] + res.results[2 * b + 1]["outp"]
                  + np.asarray(bo).astype(np.float32))
    return out


# revision 7
# speedup vs baseline: 1.0383x; 1.0188x over previous
"""Trainium2 Bass kernel for CusMultiHeadAttention.

Shapes (hardcoded): x (4,1024,1024) f32, bias (4,16,1024,1024) f32,
attention_mask (4,1024) i32, Wq/Wk/Wv (1024,1024), Wo (1024,1024), bo (1024,).

Sharding: 8 cores = 4 batches x 2 head-groups (8 heads each).
Wq/Wk/Wv column-parallel, Wo row-parallel (host sums the pair partials + bo).

Per-core pipeline (all "transposed" orientation, no on-device transposes):
  xT = x[b].T (host)                     -> SBUF (c_in on partitions)
  qT = (Wq'/8)^T @ xT, kT = Wk'^T @ xT   (feature on partitions, seq free)
  v  = x[b] @ Wv'                        (seq on partitions, feature free)
  v_aug[h] = [v[h] * mask | mask]        (mask folded into V + ones-column)
  sT[h,kt] = kT[h,kt].T @ qT[h]          ([128,512]x2 into a 6-slot PSUM ring)
  pT = exp(sT) * expbT                   (exp on Act over [128,1024] slot
                                          pairs; host-precomputed exp(bias)
                                          folded in via a bf16 DVE mul)
  o_aug[h] = sum_kt v_aug[h,kt].T @ pT   (rows 0..63 = o.T, row 64 = denom)
  o_aug is copied to SBUF immediately (frees the single PSUM accumulator for
  the next head); the denom recip/broadcast/normalize runs from SBUF on
  DVE+Pool.  Head pairs (2j, 2j+1) stack into oT_pack [128, 4, S] (odd head
  via SBUF->SBUF DMA hop) so the output projection contracts K=128:
  outp = sum_j oT_pack[:,j].T @ wo_pack[:,j]  -> DRAM partial (host adds bo)

HAM note: the PE clock-gates to 1.2 GHz unless the matmul stream is dense;
warm-up matmuls cover the input-DMA head and the phases are ordered so the
PE never idles > ~2us.
"""

import sys

if "/opt/trn_rl_repo" not in sys.path:
    sys.path.insert(0, "/opt/trn_rl_repo")

import math
from contextlib import ExitStack

import numpy as np

import concourse.mybir as mybir
import concourse.tile as tile
from concourse import bacc
from concourse.bass_utils import run_bass_kernel_spmd

B, S, C_IN = 4, 1024, 1024
N_HEAD, C = 16, 64
N_CORES = 8
HG = 8  # heads per core
F = HG * C  # 512 local features
P = 128
KT = C_IN // P  # 8 contraction tiles for projections
ST = S // P  # 8 seq tiles
VW = C + 1  # 65: v columns + ones-column
NSLOT = 6  # PSUM score ring slots (3 kt-pairs in flight)
N_WARM = 40  # HAM warm-up matmuls covering the input-DMA head
EXPB_BUFS = 4

f32 = mybir.dt.float32
bf16 = mybir.dt.bfloat16

HEAD_ORDER = [1, 2, 3, 4, 5, 6, 7, 0]  # last head is LO of its pair
PAIR_ORDER = [1, 2, 3, 0]  # out-proj accumulation order: last-ready last


def build_program():
    nc = bacc.Bacc("TRN2", target_bir_lowering=False, debug=False,
                   num_devices=N_CORES)

    xT = nc.dram_tensor("xT", (C_IN, S), bf16, kind="ExternalInput").ap()
    wq = nc.dram_tensor("wq", (C_IN, F), bf16, kind="ExternalInput").ap()
    wk = nc.dram_tensor("wk", (C_IN, F), bf16, kind="ExternalInput").ap()
    wv = nc.dram_tensor("wv", (C_IN, F), bf16, kind="ExternalInput").ap()
    # packed head pairs: rows 0:64 = head 2j, rows 64:128 = head 2j+1
    wo = nc.dram_tensor("wo", (P, HG // 2, C_IN), bf16,
                        kind="ExternalInput").ap()
    expbT = nc.dram_tensor("expbT", (HG, S, S), bf16,
                           kind="ExternalInput").ap()
    maskf = nc.dram_tensor("maskf", (S,), f32, kind="ExternalInput").ap()
    outp = nc.dram_tensor("outp", (S, C_IN), f32, kind="ExternalOutput").ap()

    with tile.TileContext(nc) as tc:
        with ExitStack() as ctx:
            persist = ctx.enter_context(tc.tile_pool(name="persist", bufs=1))
            mask_sb = persist.tile([P, ST], f32)
            ones_sb = persist.tile([P, HG, 1], f32)
            v_sb = persist.tile([P, ST, HG * VW], bf16)
            qT_sb = persist.tile([P, F // P, S], bf16)
            kT_sb = persist.tile([P, F // P, S], bf16)
            wo_sb = persist.tile([P, HG // 2, C_IN], bf16)
            oT_pack = persist.tile([P, HG // 2, S], bf16)
            oT_hi = persist.tile([C, HG // 2, S], bf16)
            warm_sb = persist.tile([P, 640], bf16)

            xT_sb = persist.tile([P, KT, S], bf16)
            wq_sb = persist.tile([P, KT, F], bf16)
            wk_sb = persist.tile([P, KT, F], bf16)
            wv_sb = persist.tile([P, KT, F], bf16)

            # gpsimd queue: everything the projections need, wq/xT first.
            nc.gpsimd.memset(warm_sb[:], 0.25)
            nc.gpsimd.dma_start(wq_sb[:],
                                wq.rearrange("(kt p) f -> p kt f", p=P))
            for kt in range(KT):
                nc.gpsimd.dma_start(xT_sb[:, kt, :], xT[kt * P:(kt + 1) * P, :])
            nc.gpsimd.dma_start(wk_sb[:],
                                wk.rearrange("(kt p) f -> p kt f", p=P))
            nc.gpsimd.dma_start(wv_sb[:],
                                wv.rearrange("(kt p) f -> p kt f", p=P))
            nc.gpsimd.dma_start(wo_sb[:], wo)
            nc.gpsimd.dma_start(mask_sb[:],
                                maskf.rearrange("(t p) -> p t", p=P))
            nc.gpsimd.memset(ones_sb[:], 1.0)

            # sync queue: dedicated exp(bias) stream, 4 heads deep
            expb_pool = ctx.enter_context(tc.tile_pool(name="expb",
                                                       bufs=EXPB_BUFS))

            def expb_dma(h):
                t = expb_pool.tile([P, ST, S], bf16, name=f"expb{h}",
                                   tag="expb")
                nc.sync.dma_start(t[:],
                                  expbT[h].rearrange("(kt p) s -> p kt s",
                                                     p=P))
                return t

            expb_tiles = {h: expb_dma(h) for h in HEAD_ORDER[:EXPB_BUFS]}

            # ---- HAM warm-up: keep the PE busy while inputs stream in ----
            with tc.tile_pool(name="psW", bufs=1, space="PSUM") as psW:
                wps = psW.tile([P, 512], f32, name="wps")
                for _ in range(N_WARM):
                    nc.tensor.matmul(wps[:], warm_sb[:, 0:P],
                                     warm_sb[:, P:P + 512],
                                     start=True, stop=True)

            # ---- phase P: projections ----
            with tc.tile_pool(name="psProj", bufs=2, space="PSUM") as psProj, \
                 tc.tile_pool(name="psV", bufs=2, space="PSUM") as psV:
                for mt in range(F // P):
                    for w_sb, dst in ((wq_sb, qT_sb), (wk_sb, kT_sb)):
                        ps = psProj.tile([P, S], f32, name="ps_proj")
                        for nh in range(2):
                            for kt in range(KT):
                                nc.tensor.matmul(
                                    ps[:, nh * 512:(nh + 1) * 512],
                                    w_sb[:, kt, mt * P:(mt + 1) * P],
                                    xT_sb[:, kt, nh * 512:(nh + 1) * 512],
                                    start=(kt == 0), stop=(kt == KT - 1))
                        nc.vector.tensor_copy(dst[:, mt, :], ps[:])

                for mt in range(ST):
                    psv = psV.tile([P, F], f32, name="psv")
                    for kt in range(KT):
                        nc.tensor.matmul(
                            psv[:],
                            xT_sb[:, kt, mt * P:(mt + 1) * P],
                            wv_sb[:, kt, :],
                            start=(kt == 0), stop=(kt == KT - 1))
                    m_col = mask_sb[:, mt:mt + 1]
                    v_view = v_sb[:, mt, :].rearrange("p (h c) -> p h c", c=VW)
                    nc.vector.tensor_scalar_mul(
                        v_view[:, :, 0:C],
                        psv.rearrange("p (h c) -> p h c", c=C), m_col)
                    nc.vector.tensor_scalar_mul(
                        v_view[:, :, C:C + 1], ones_sb[:], m_col)

            # ---- phase A: attention ----
            with tc.tile_pool(name="psRing", bufs=1, space="PSUM") as psRing, \
                 tc.tile_pool(name="psO", bufs=1, space="PSUM") as psO, \
                 tc.tile_pool(name="pe", bufs=3) as pe_pool, \
                 tc.tile_pool(name="pt", bufs=3) as pt_pool, \
                 tc.tile_pool(name="oagg", bufs=2) as oagg_pool, \
                 tc.tile_pool(name="rc0", bufs=2) as rc0_pool, \
                 tc.tile_pool(name="rcb", bufs=2) as rcb_pool:

                ring = psRing.tile([P, NSLOT, 512], f32, name="ring")
                slot = [0]  # rotating ring cursor (pairs use slot, slot+1)

                for hi_idx, h in enumerate(HEAD_ORDER):
                    po = (h % 2) * C
                    mt_h = h // 2
                    kT_h = kT_sb[po:po + C, mt_h, :]
                    qT_h = qT_sb[po:po + C, mt_h, :]
                    expb_sb = expb_tiles[h]
                    oaps = psO.tile([VW, S], f32, name="oaug")
                    pair_slot = [0] * ST
                    pair_pt = [None] * ST

                    def s_pair(kt):
                        sl = slot[0]
                        slot[0] = (slot[0] + 2) % NSLOT
                        pair_slot[kt] = sl
                        for nh in range(2):
                            nc.tensor.matmul(
                                ring[:, sl + nh, :],
                                kT_h[:, kt * P:(kt + 1) * P],
                                qT_h[:, nh * 512:(nh + 1) * 512],
                                start=True, stop=True)

                    def exp_pair(kt):
                        sl = pair_slot[kt]
                        pe_t = pe_pool.tile([P, S], bf16, name="pe", tag="pe")
                        nc.scalar.activation(
                            pe_t[:],
                            ring[:, sl:sl + 2, :].rearrange("p a b -> p (a b)"),
                            mybir.ActivationFunctionType.Exp)
                        pt_t = pt_pool.tile([P, S], bf16, name="pt", tag="pt")
                        nc.vector.tensor_mul(pt_t[:], pe_t[:],
                                             expb_sb[:, kt, :])
                        pair_pt[kt] = pt_t

                    def pv_pair(kt):
                        for nh in range(2):
                            nc.tensor.matmul(
                                oaps[:, nh * 512:(nh + 1) * 512],
                                v_sb[:, kt, h * VW:(h + 1) * VW],
                                pair_pt[kt][:, nh * 512:(nh + 1) * 512],
                                start=(kt == 0), stop=(kt == ST - 1))

                    # software pipeline: S-pairs run 2 ahead of PV-pairs
                    s_pair(0)
                    exp_pair(0)
                    s_pair(1)
                    exp_pair(1)
                    for kt in range(2, ST):
                        s_pair(kt)
                        pv_pair(kt - 2)
                        exp_pair(kt)
                    pv_pair(ST - 2)
                    pv_pair(ST - 1)

                    # prefetch exp(bias) for a later head into the freed slot
                    if hi_idx + EXPB_BUFS < HG:
                        nh_ = HEAD_ORDER[hi_idx + EXPB_BUFS]
                        expb_tiles[nh_] = expb_dma(nh_)

                    # free the PSUM accumulator fast, then normalize from SBUF
                    oagg = oagg_pool.tile([VW, S], f32, name="oagg")
                    nc.vector.tensor_copy(oagg[:], oaps[:])
                    rc0 = rc0_pool.tile([1, S], f32, name="rc0")
                    nc.gpsimd.dma_start(rc0[:], oagg[C:C + 1, :])
                    rcv = rc0_pool.tile([1, S], f32, name="rcv", tag="rcv")
                    nc.vector.reciprocal_approx_fast(rcv[:], rc0[:])
                    rcb = rcb_pool.tile([C, S], f32, name="rcb")
                    nc.gpsimd.partition_broadcast(rcb[:], rcv[:])
                    j = h // 2
                    if h % 2 == 0:
                        nc.gpsimd.tensor_mul(oT_pack[0:C, j, :],
                                             oagg[0:C, :], rcb[:])
                    else:
                        nc.gpsimd.tensor_mul(oT_hi[:, j, :],
                                             oagg[0:C, :], rcb[:])
                        nc.gpsimd.dma_start(oT_pack[C:P, j, :],
                                            oT_hi[:, j, :])

            # ---- phase O: output projection (row-parallel partial) ----
            with tc.tile_pool(name="outsb", bufs=3) as out_pool, \
                 tc.tile_pool(name="psOut", bufs=2, space="PSUM") as psOut:
                for qt in range(ST):
                    for nh in range(2):
                        pso = psOut.tile([P, 512], f32, name="pso")
                        for i, j in enumerate(PAIR_ORDER):
                            nc.tensor.matmul(
                                pso[:],
                                oT_pack[:, j, qt * P:(qt + 1) * P],
                                wo_sb[:, j, nh * 512:(nh + 1) * 512],
                                start=(i == 0), stop=(i == HG // 2 - 1))
                        osb = out_pool.tile([P, 512], f32, name="osb")
                        nc.scalar.copy(osb[:], pso[:])
                        nc.gpsimd.dma_start(
                            outp[qt * P:(qt + 1) * P,
                                 nh * 512:(nh + 1) * 512],
                            osb[:])

    nc.compile()
    return nc


def make_in_maps(x, bias, attention_mask, Wq, Wk, Wv, Wo):
    import ml_dtypes
    bf = ml_dtypes.bfloat16
    scale = 1.0 / math.sqrt(C)
    wq_scaled = (np.asarray(Wq) * scale).astype(bf)
    x = np.asarray(x)
    bias = np.asarray(bias)
    wk16 = np.asarray(Wk).astype(bf)
    wv16 = np.asarray(Wv).astype(bf)
    wo = np.asarray(Wo)
    in_maps = []
    for c in range(N_CORES):
        b, hg = c // 2, c % 2
        fs = slice(hg * F, (hg + 1) * F)
        # pack Wo rows as head pairs (2j, 2j+1) stacked along partitions
        wo_l = wo[fs, :].reshape(HG, C, C_IN)
        wo_pack = np.concatenate(
            [np.concatenate([wo_l[2 * j], wo_l[2 * j + 1]], axis=0)[None]
             for j in range(HG // 2)], axis=0)  # (4, 128, C_IN)
        wo_pack = np.ascontiguousarray(
            wo_pack.transpose(1, 0, 2)).astype(bf)  # (128, 4, C_IN)
        expb = np.exp(
            bias[b, hg * HG:(hg + 1) * HG].transpose(0, 2, 1)).astype(bf)
        in_maps.append({
            "xT": np.ascontiguousarray(x[b].T.astype(bf)),
            "wq": np.ascontiguousarray(wq_scaled[:, fs]),
            "wk": np.ascontiguousarray(wk16[:, fs]),
            "wv": np.ascontiguousarray(wv16[:, fs]),
            "wo": wo_pack,
            "expbT": np.ascontiguousarray(expb),
            "maskf": np.asarray(attention_mask)[b].astype(np.float32),
        })
    return in_maps


_NC_CACHE = []


def get_program():
    if not _NC_CACHE:
        _NC_CACHE.append(build_program())
    return _NC_CACHE[0]


def run(in_maps, trace=False, **kw):
    nc = get_program()
    return run_bass_kernel_spmd(nc, in_maps, core_ids=list(range(N_CORES)),
                                trace=trace, **kw)


def kernel(x, bias, attention_mask, Wq, Wk, Wv, Wo, bo):
    in_maps = make_in_maps(x, bias, attention_mask, Wq, Wk, Wv, Wo)
    res = run(in_maps)
    out = np.empty((B, S, C_IN), dtype=np.float32)
    for b in range(B):
        out[b] = (res.results[2 * b]["outp"] + res.results[2 * b + 1]["outp"]
                  + np.asarray(bo).astype(np.float32))
    return out
